# revision 11
# baseline (speedup 1.0000x reference)
"""Trainium2 Bass kernel for nn_ColorNet: 7x7 box conv s2 -> 3x3 maxpool s2 ->
27 sequential 3x3 box convs (strides [1]*6+[2]+[1]*8+[2]+[1]*11).

Decomposition (exact linear algebra, validated separately):
  - every 2D stage is separable: stage(X) = Bv @ X @ Bh^T
  - conv1:  Y1 = B1 @ X @ B1^T  with B1 = 256x512 banded 7-tap stride-2 matrix
  - maxpool: separable 3-window stride-2 max along each axis
  - the 27-conv tail is linear: collapses to  F = M @ Z @ M^T, M = 32x128

Mapping (per core, 16 images, data parallel across 8 cores):
  C_v   : 6 banded matmuls (f32r) contracting H          -> [h' x w]   psum
  T1    : 8 PE transposes (128x128)                       -> [w x h']
  C_h   : 6 banded matmuls (f32r) contracting W           -> [w' x h']  psum
  M_v   : max along h' (free dim), reduce+tensor_max trick
  T2    : 2 PE transposes                                  -> [h'' x w']
  M_h   : max along w' (free dim)                          -> [h'' x w'']
  VB/T3/FB: M @ Z, transpose, (M Z)^T-style final matmul  -> [32 x 32]
"""
import numpy as np

N_CORES = 8
N_IMGS = 128
PER_CORE = N_IMGS // N_CORES  # 16

_HCS = [[0, 1, 2], [1, 2, 3]]  # contributing input chunks per output chunk

_STRIDES_3x3 = [1] * 6 + [2] + [1] * 8 + [2] + [1] * 11


def _conv_matrix(n_in, taps, s, p, dtype=np.float64):
    k = len(taps)
    n_out = (n_in + 2 * p - k) // s + 1
    A = np.zeros((n_out, n_in), dtype=dtype)
    for i in range(n_out):
        for j in range(k):
            idx = s * i + j - p
            if 0 <= idx < n_in:
                A[i, idx] = taps[j]
    return A


def _rank1_taps(f2d):
    """Split a (separable) 2D kernel into vertical/horizontal 1D taps."""
    f = np.asarray(f2d, dtype=np.float64)
    u, s, vt = np.linalg.svd(f)
    assert s[0] > 0 and (len(s) == 1 or s[1] < 1e-6 * s[0]), "filter not rank-1"
    kv = u[:, 0] * np.sqrt(s[0])
    kh = vt[0, :] * np.sqrt(s[0])
    if kv.sum() < 0:  # fix sign convention
        kv, kh = -kv, -kh
    return kv, kh


def _host_matrices(filter1, filter2):
    """Build the constant matrices shipped to every core (float32)."""
    kv1, kh1 = _rank1_taps(filter1[0, 0])
    kv2, kh2 = _rank1_taps(filter2[0, 0])
    B1v = _conv_matrix(512, kv1, 2, 3)
    B1h = _conv_matrix(512, kh1, 2, 3)

    def tail(taps):
        n = 128
        M = np.eye(n)
        for s in _STRIDES_3x3:
            A = _conv_matrix(n, taps, s, 1)
            M = A @ M
            n = A.shape[0]
        return M  # 32x128

    Mv = tail(kv2)
    Mh = tail(kh2)

    # 6 128x128 blocks of B^T used by both C_v (B1v) and C_h (B1h)
    def blocks(B):
        BT = B.T  # 512x256
        out = np.zeros((6, 128, 128))
        acc = np.zeros_like(BT)
        for oc in range(2):
            for j, hc in enumerate(_HCS[oc]):
                blk = BT[128 * hc:128 * (hc + 1), 128 * oc:128 * (oc + 1)]
                out[3 * oc + j] = blk
                acc[128 * hc:128 * (hc + 1), 128 * oc:128 * (oc + 1)] += blk
        assert np.array_equal(acc, BT), "banded blocks do not cover B^T"
        return out

    consts = {
        "wcv": blocks(B1v).astype(np.float32),
        "wch": blocks(B1h).astype(np.float32),
        "wsegv": Mv.T.astype(np.float32).copy(),  # [128, 32]
        "wsegh": Mh.T.astype(np.float32).copy(),  # [128, 32]
        "id128": np.eye(128, dtype=np.float32),
        "id32": np.eye(32, dtype=np.float32),
    }
    return consts


_NC_CACHE = {}


def _build_nc(reps=1, dma_only=False):
    key = (reps, dma_only)
    if key in _NC_CACHE:
        return _NC_CACHE[key]
    import concourse.bass as bass
    import concourse.tile as tile
    from concourse import bacc, mybir

    f32 = mybir.dt.float32
    f32r = mybir.dt.float32r

    nc = bacc.Bacc("TRN2", target_bir_lowering=False, debug=False,
                   num_devices=N_CORES)
    x_d = nc.dram_tensor("x", [PER_CORE, 1, 512, 512], f32r,
                         kind="ExternalInput").ap()
    wcv_d = nc.dram_tensor("wcv", [6, 128, 128], f32r, kind="ExternalInput").ap()
    wch_d = nc.dram_tensor("wch", [6, 128, 128], f32r, kind="ExternalInput").ap()
    wsegv_d = nc.dram_tensor("wsegv", [128, 32], f32, kind="ExternalInput").ap()
    wsegh_d = nc.dram_tensor("wsegh", [128, 32], f32, kind="ExternalInput").ap()
    id128_d = nc.dram_tensor("id128", [128, 128], f32r, kind="ExternalInput").ap()
    id32_d = nc.dram_tensor("id32", [32, 32], f32, kind="ExternalInput").ap()
    y_d = nc.dram_tensor("y", [PER_CORE, 1, 32, 32], f32,
                         kind="ExternalOutput").ap()

    with tile.TileContext(nc) as tc:
        with (
            tc.tile_pool(name="consts", bufs=1) as cpool,
            tc.tile_pool(name="x", bufs=6) as xpool,
            tc.tile_pool(name="y1", bufs=3) as y1pool,
            tc.tile_pool(name="t1", bufs=3) as t1pool,
            tc.tile_pool(name="mv", bufs=3) as mvpool,
            tc.tile_pool(name="mh", bufs=2) as mhpool,
            tc.tile_pool(name="small", bufs=2) as smpool,
            tc.tile_pool(name="cvp", bufs=2, space="PSUM") as cvppool,
            tc.tile_pool(name="t1p", bufs=2, space="PSUM") as t1ppool,
            tc.tile_pool(name="chp", bufs=3, space="PSUM") as chppool,
            tc.tile_pool(name="t2segp", bufs=1, space="PSUM") as t2segppool,
        ):
            # --- load constants once ---
            wcv = cpool.tile([128, 6 * 128], f32r, tag="wcv")
            nc.sync.dma_start(wcv[:].rearrange("p (b m) -> p b m", b=6),
                              wcv_d.rearrange("b p m -> p b m"))
            wch = cpool.tile([128, 6 * 128], f32r, tag="wch")
            nc.sync.dma_start(wch[:].rearrange("p (b m) -> p b m", b=6),
                              wch_d.rearrange("b p m -> p b m"))
            wsegv = cpool.tile([128, 32], f32, tag="wsegv")
            nc.sync.dma_start(wsegv[:], wsegv_d)
            wsegh = cpool.tile([128, 32], f32, tag="wsegh")
            nc.sync.dma_start(wsegh[:], wsegh_d)
            id128 = cpool.tile([128, 128], f32r, tag="id128")
            nc.sync.dma_start(id128[:], id128_d)
            id32 = cpool.tile([32, 32], f32, tag="id32")
            nc.sync.dma_start(id32[:], id32_d)

            outs_all = smpool.tile([32, 32 * PER_CORE], f32, tag="outsall")
            import contextlib
            loop_cm = (tc.For_i(0, reps, 1) if reps > 1
                       else contextlib.nullcontext())
            with loop_cm:
              for n in range(PER_CORE):
                # ---- load image: [128 p(h%128), (hc, w)] ----
                xt = xpool.tile([128, 2048], f32r, tag="xt")
                xv = x_d[n, 0].rearrange("(c p) w -> p c w", p=128)
                for c4 in range(4):
                    nc.sync.dma_start(
                        xt[:, 512 * c4:512 * (c4 + 1)],
                        xv[:, c4])

                if dma_only:
                    outt = smpool.tile([32, 32], f32, tag="outt")
                    nc.vector.tensor_copy(outt[:], xt[0:32, 0:32].bitcast(f32))
                    nc.sync.dma_start(y_d[n, 0], outt[:])
                    continue
                # ---- C_v: 6 banded matmuls, contract H ----
                y1 = y1pool.tile([128, 1024], f32r, tag="y1")
                for oc in range(2):
                    cvp = cvppool.tile([128, 512], f32, tag="cvp")
                    for j, hc in enumerate(_HCS[oc]):
                        nc.tensor.matmul(
                            cvp[:],
                            wcv[:, 128 * (3 * oc + j):128 * (3 * oc + j + 1)],
                            xt[:, 512 * hc:512 * (hc + 1)],
                            start=(j == 0), stop=(j == 2))
                    # psum -> sbuf with f32r rounding (split DVE/ACT)
                    if oc == 0:
                        nc.vector.tensor_copy(y1[:, 0:512], cvp[:])
                    else:
                        nc.scalar.copy(y1[:, 512:1024], cvp[:])

                # ---- T1: transpose 8 blocks of [h' x w] -> [w x h'] ----
                t1 = t1pool.tile([128, 1024], f32r, tag="t1")
                for half in range(2):
                    t1p = t1ppool.tile([128, 512], f32r, tag="t1p")
                    for ws2 in range(2):
                        ws = 2 * half + ws2
                        for hc in range(2):
                            nc.tensor.transpose(
                                t1p[:, 256 * ws2 + 128 * hc:256 * ws2 + 128 * (hc + 1)],
                                y1[:, 512 * hc + 128 * ws:512 * hc + 128 * (ws + 1)],
                                id128[:])
                    nc.scalar.copy(t1[:, 512 * half:512 * (half + 1)], t1p[:])

                # ---- C_h: 6 banded matmuls, contract W -> [w' x h'] ----
                chps = []
                for wc in range(2):
                    chp = chppool.tile([128, 256], f32, tag="chp")
                    chps.append(chp)
                    for j, ws in enumerate(_HCS[wc]):
                        nc.tensor.matmul(
                            chp[:],
                            wch[:, 128 * (3 * wc + j):128 * (3 * wc + j + 1)],
                            t1[:, 256 * ws:256 * (ws + 1)],
                            start=(j == 0), stop=(j == 2))

                # ---- M_v: max over h' windows (free dim) ----
                mv = mvpool.tile([128, 256], f32, tag="mv")
                # chunk 0 on DVE, straight from PSUM
                v0 = chps[0][:].rearrange("p (j k) -> p j k", k=2)
                nc.vector.tensor_reduce(mv[:, 0:128], v0,
                                        axis=mybir.AxisListType.X,
                                        op=mybir.AluOpType.max)
                nc.vector.tensor_max(mv[:, 1:128], mv[:, 1:128],
                                     v0[:, 0:127, 1])
                # chunk 1 likewise on DVE (Pool codegen rejects TensorTensor)
                v1 = chps[1][:].rearrange("p (j k) -> p j k", k=2)
                nc.vector.tensor_reduce(mv[:, 128:256], v1,
                                        axis=mybir.AxisListType.X,
                                        op=mybir.AluOpType.max)
                nc.vector.tensor_max(mv[:, 129:256], mv[:, 129:256],
                                     v1[:, 0:127, 1])

                # ---- T2: [w' x h''] -> [h'' x w'] ----
                t2seg = t2segppool.tile([128, 448], f32, tag="t2seg")
                t2p = t2seg[:, 0:256]
                for wc in range(2):
                    nc.tensor.transpose(t2p[:, 128 * wc:128 * (wc + 1)],
                                        mv[:, 128 * wc:128 * (wc + 1)],
                                        id128[:].bitcast(f32))

                # ---- M_h: max over w' windows (free dim) ----
                mh = mhpool.tile([128, 128], f32, tag="mh")
                h0 = t2p[:].rearrange("p (j k) -> p j k", k=2)
                nc.vector.tensor_reduce(mh[:, 0:128], h0,
                                        axis=mybir.AxisListType.X,
                                        op=mybir.AluOpType.max)
                nc.vector.tensor_max(mh[:, 1:128], mh[:, 1:128],
                                     h0[:, 0:127, 1])

                # ---- tail: F = Mv @ Z @ Mh^T via VB, T3, FB ----
                segp = t2seg[:, 256:448]
                nc.tensor.matmul(segp[0:32, 0:128], wsegv[:], mh[:],
                                 start=True, stop=True)
                vb = smpool.tile([32, 128], f32, tag="vb")
                nc.scalar.copy(vb[:], segp[0:32, 0:128])
                nc.tensor.transpose(segp[:, 128:160], vb[:], id32[:])
                t3 = smpool.tile([128, 32], f32, tag="t3")
                nc.vector.tensor_copy(t3[:], segp[:, 128:160])
                nc.tensor.matmul(segp[0:32, 160:192], t3[:], wsegh[:],
                                 start=True, stop=True)
                nc.scalar.copy(outs_all[:, 32 * n:32 * (n + 1)],
                               segp[0:32, 160:192])
              nc.sync.dma_start(
                  y_d[:, 0].rearrange("n h w -> h n w"),
                  outs_all[:].rearrange("h (n w) -> h n w", w=32))

    nc.compile()
    _NC_CACHE[key] = nc
    return nc


def kernel(x, filter1, filter2):
    from concourse.bass_utils import run_bass_kernel_spmd

    x = np.ascontiguousarray(np.asarray(x, dtype=np.float32))
    assert x.shape == (N_IMGS, 1, 512, 512)
    consts = _host_matrices(np.asarray(filter1), np.asarray(filter2))
    nc = _build_nc()
    in_maps = []
    for c in range(N_CORES):
        m = {"x": x[c * PER_CORE:(c + 1) * PER_CORE]}
        m.update(consts)
        in_maps.append(m)
    res = run_bass_kernel_spmd(nc, in_maps, list(range(N_CORES)))
    y = np.concatenate([res.results[c]["y"] for c in range(N_CORES)], axis=0)
    return y.astype(np.float32)
